# revision 89
# baseline (speedup 1.0000x reference)
"""DGCNN (4x DynamicEdgeConv + global head) Trainium2 Bass kernel, v3.

Sharding: data-parallel over the 16 clouds -> 8 cores x 2 clouds; tiny BN
head on host (cross-core batch stats).

v3 redesign (vs the 747us v2): the v2 kernel was DVE-bound (95% busy:
top-k Max/MatchReplace/pack 447us + K-max TensorReduce 214us).
  - pack-free top-k: Act evicts gram PSUM scores as fp16 into the HIGH
    int16 lane of a [128, N] fp32 tile whose LOW lanes hold a prewritten
    int16 iota.  The fp32 bit pattern [fp16(score)][iota] is monotone in
    score (float compare; NaN patterns unreachable), so Max/MatchReplace
    run directly on it and the low lane of a winner IS its column index.
    This deletes the DVE pack pass and reuses the evict Act was doing
    anyway.
  - chunked top-k: top-8 per 128-col chunk (8x Max) + top-20-of-64
    second level (Max/MR/Max/MR/Max over 64) instead of 3x Max + 2x
    MatchReplace over the full 1024 cols: 2.1us -> 0.9us per t-block on
    DVE.  (P(>8 of the 20 NN in one 128-chunk) ~ 3e-4 per point; a miss
    swaps a ~20th neighbor for a ~21st.)
  - fp16 U/V tables everywhere; u+v add as fp16 TensorTensor (2x DVE
    rate, or Pool for load balance); relu on Act; L3 gather runs d=1 on
    the fp32-bitcast pair table (half the modeled gather cost).
  - L0/L1 h2 matmuls use a host-built block-diagonal [128,128] Wb so the
    two 64-point groups multiply in one matmul.
"""
import numpy as np

import concourse.bacc as bacc
import concourse.mybir as mybir
import concourse.tile as tile
from concourse.bass_utils import run_bass_kernel_spmd

dt = mybir.dt
AF = mybir.ActivationFunctionType
OP = mybir.AluOpType
AX = mybir.AxisListType

B, N, K = 16, 1024, 20
NCORES = 8
CPC = B // NCORES                    # clouds per core
LAYERS = [(3, 64), (64, 64), (64, 128), (128, 256)]
ECHUNK = 1280                        # idxs per ap_gather
SCALE = 0.0625                       # fp16 score scale (overflow guard)

_NC_CACHE = {}


def _build_nc():
    if "nc" in _NC_CACHE:
        return _NC_CACHE["nc"]
    nc = bacc.Bacc("TRN2", target_bir_lowering=False, debug=False,
                   num_devices=NCORES)

    # ---------------- DRAM I/O ----------------
    posT = nc.dram_tensor("posT", [CPC, 3, N], dt.float32,
                          kind="ExternalInput").ap()
    sqh0_in = nc.dram_tensor("sqh0", [CPC, 1, N], dt.float32,
                             kind="ExternalInput").ap()
    sqh0T_in = nc.dram_tensor("sqh0T", [CPC, 128, 8], dt.float32,
                              kind="ExternalInput").ap()
    wau, wav, wba, wb, wbb = [], [], [], [], []
    for li, (C, Co) in enumerate(LAYERS):
        packed = (Co == 64)
        wau.append(nc.dram_tensor(f"wau{li}", [C, Co], dt.float32, kind="ExternalInput").ap())
        wav.append(nc.dram_tensor(f"wav{li}", [C, Co], dt.float32, kind="ExternalInput").ap())
        wba.append(nc.dram_tensor(f"wba{li}", [Co, 1], dt.float32, kind="ExternalInput").ap())
        wbshape = [128, 128] if packed else [Co, Co]
        wb.append(nc.dram_tensor(f"wb{li}", wbshape, dt.float16, kind="ExternalInput").ap())
        wbb.append(nc.dram_tensor(f"wbb{li}", [Co, 1], dt.float32, kind="ExternalInput").ap())
    lin1 = nc.dram_tensor("lin1", [512, 1024], dt.float32, kind="ExternalInput").ap()
    lin1b = nc.dram_tensor("lin1b", [128, 8], dt.float32, kind="ExternalInput").ap()
    h_out = nc.dram_tensor("h_out", [128, 8 * CPC], dt.float32,
                           kind="ExternalOutput").ap()

    with tile.TileContext(nc) as tc:
        with (
            tc.tile_pool(name="const", bufs=1) as cpool,
            tc.tile_pool(name="xf", bufs=1) as xpool,
            tc.tile_pool(name="uv", bufs=2) as uvpool,
            tc.tile_pool(name="small", bufs=2) as smpool,
            tc.tile_pool(name="idx", bufs=2) as idxpool,
            tc.tile_pool(name="vg", bufs=4) as vgpool,
            tc.tile_pool(name="ps_g", bufs=2, space="PSUM") as ps_g,
            tc.tile_pool(name="ps_uv", bufs=2, space="PSUM") as ps_uv,
            tc.tile_pool(name="ps_h2", bufs=2, space="PSUM") as ps_h2,
        ):
            # ------------- constants -------------
            masklo16 = cpool.tile([128, 1], dt.int16, tag="masklo16")
            nc.vector.memset(masklo16[:], 1023)
            nh0 = cpool.tile([128, 1], dt.float32, tag="nh0")
            nc.vector.memset(nh0[:], -0.5)
            nh = cpool.tile([128, 1], dt.float32r, tag="nh")
            nc.scalar.activation(nh[:], nh0[:], AF.Copy)
            nh2 = cpool.tile([128, 2], dt.float32r, tag="nh2")
            nc.scalar.activation(nh2[:], nh0[:].to_broadcast([128, 2]), AF.Copy)
            ones0 = cpool.tile([1, 128], dt.float32, tag="ones0")
            nc.vector.memset(ones0[:], 1.0)
            ones1 = cpool.tile([1, 128], dt.float32r, tag="ones1")
            nc.scalar.activation(ones1[:], ones0[:], AF.Copy)

            # per-cloud feature tiles (f32r) - load first so layer 0 can start
            xts_c = {}
            sqh_sb = {}
            sqh0Ts_sb = {}
            for cl in range(CPC):
                x0 = xpool.tile([3, N], dt.float32r, tag=f"x0_c{cl}", name=f"x0c{cl}")
                nc.sync.dma_start(x0.copy()[:], posT[cl].bitcast(dt.float32r))
                xts_c[cl] = [[x0]]
                s0 = smpool.tile([1, N], dt.float32r, tag=f"sqh_c{cl}", bufs=1,
                                 name=f"sqh0c{cl}")
                nc.sync.dma_start(s0.copy()[:], sqh0_in[cl].bitcast(dt.float32r))
                sqh_sb[cl] = s0
                s0T = smpool.tile([128, 8], dt.float32, tag=f"sqh0Ts_c{cl}",
                                  bufs=1, name=f"sqh0Tc{cl}")
                nc.sync.dma_start(s0T.copy()[:], sqh0T_in[cl])
                sqh0Ts_sb[cl] = s0T

            # packed-score tiles: low int16 lanes = iota (written on-chip by
            # the otherwise-idle Pool engine), high lanes get fp16 scores
            # from Act.  Two buffers, rotated per t.
            pks = []
            for pb in range(2):
                pk = cpool.tile([128, N], dt.float32, tag=f"pk{pb}",
                                name=f"pk{pb}")
                pk16i = pk[:].bitcast(dt.int16).rearrange(
                    "f (e two) -> f e two", two=2)
                nc.gpsimd.iota(pk16i[:, :, 0], pattern=[[1, N]], base=0,
                               channel_multiplier=0)
                pks.append(pk)

            # weight tiles; DMAs are deferred to one layer ahead of use so
            # the startup HWDGE queue doesn't gate the first topk
            wau_sb, wav_sb, wba_sb, wb_sb, wbb_sb = [], [], [], [], []
            wload = []
            for li, (C, Co) in enumerate(LAYERS):
                packed = (Co == 64)
                dmas = []
                t = cpool.tile([C, Co], dt.float32r, tag=f"wau{li}")
                dmas.append((t[:], wau[li][:].bitcast(dt.float32r))); wau_sb.append(t)
                t = cpool.tile([C, Co], dt.float32r, tag=f"wav{li}")
                dmas.append((t[:], wav[li][:].bitcast(dt.float32r))); wav_sb.append(t)
                nob = (Co + 127) // 128
                bblks = []
                for ob in range(nob):
                    kk = min(128, Co - ob * 128)
                    t = cpool.tile([kk, 1], dt.float32, tag=f"wba{li}_{ob}")
                    dmas.append((t[:], wba[li][ob * 128: ob * 128 + kk, :]))
                    bblks.append(t)
                wba_sb.append(bblks)
                if li == 0:
                    # layer-0 UV weights must load upfront; its B weights can
                    # wait until the A(0,0) idxw unit with the layer-1 batch
                    wload.append(list(dmas))
                    dmas.clear()
                if packed:
                    # host-built block-diagonal [128, 128]
                    t = cpool.tile([128, 128], dt.float16, tag=f"wb{li}")
                    dmas.append((t[:], wb[li][:]))
                    wb_sb.append([t])
                    t = cpool.tile([128, 1], dt.float32, tag=f"wbb{li}")
                    dmas.append((t[0:64, :], wbb[li][:]))
                    dmas.append((t[64:128, :], wbb[li][:]))
                    wbb_sb.append([t])
                else:
                    nkb = (Co + 127) // 128
                    blks = []
                    for kb in range(nkb):
                        kk = min(128, Co - kb * 128)
                        t = cpool.tile([kk, Co], dt.float16, tag=f"wb{li}_{kb}")
                        dmas.append((t[:], wb[li][kb * 128: kb * 128 + kk, :]))
                        blks.append(t)
                    wb_sb.append(blks)
                    bblks = []
                    for ob in range(nob):
                        kk = min(128, Co - ob * 128)
                        t = cpool.tile([kk, 1], dt.float32, tag=f"wbb{li}_{ob}")
                        dmas.append((t[:], wbb[li][ob * 128: ob * 128 + kk, :]))
                        bblks.append(t)
                    wbb_sb.append(bblks)
                wload.append(dmas)

            # wload groups: [UV0], [B0], [L1], [L2], [L3]
            _wsched = {0: [0], 1: [1, 2], 2: [3], 3: [4], 4: []}
            wloaded = set()

            def load_weights(step):
                for g in _wsched.get(step, []):
                    if g in wloaded or g >= len(wload):
                        continue
                    wloaded.add(g)
                    for dst, src in wload[g]:
                        nc.sync.dma_start(dst, src)

            load_weights(0)
            pooled = cpool.tile([128, 4 * CPC], dt.float32, tag="pooled")

            stateA = {}
            tctr = [0]          # global t-block counter for pk rotation

            def phaseA_units(li, cl):
                C, Co = LAYERS[li]
                packed = (Co == 64)
                nob = (Co + 127) // 128
                xt_blocks = xts_c[cl][li]      # input blocks [cb, N] f32r
                nkb_in = len(xt_blocks)
                cbs = [min(128, C - kb * 128) for kb in range(nkb_in)]
                # ---- sq pipeline: sqh = -0.5|x|^2 (f32r) ----
                if li == 0:
                    sqh = sqh_sb[cl]
                else:
                    xsqs = []
                    for kb, xb in enumerate(xt_blocks):
                        xsq = smpool.tile([cbs[kb], N], dt.float32r, tag=f"xsq{kb}",
                                          name=f"xsq{kb}")
                        nc.scalar.activation(xsq[:], xb[:].bitcast(dt.float32), AF.Square)
                        xsqs.append(xsq)
                    sq_ps = []
                    for j in range(2):
                        sl = slice(j * 512, (j + 1) * 512)
                        sp = ps_uv.tile([1, 512], dt.float32, tag="uvps", name=f"sqps{j}")
                        for kb, xsq in enumerate(xsqs):
                            nc.tensor.matmul(sp[:], nh[0:cbs[kb], :], xsq[:, sl],
                                             start=(kb == 0), stop=(kb == nkb_in - 1))
                        sq_ps.append(sp)
                    sqh = smpool.tile([1, N], dt.float32r, tag=f"sqh_c{cl}", bufs=1,
                                      name=f"sqh{li}_{cl}")
                    for j in range(2):
                        sl = slice(j * 512, (j + 1) * 512)
                        nc.scalar.activation(sqh[:, sl], sq_ps[j][:], AF.Identity)
                # per-row bias = sqh_i * SCALE so the evicted fp16 score is
                # ~ -d_ij/2 * SCALE: top scores sit near 0 where fp16
                # resolution is relative to each distance.  sqhT[p, t] =
                # -0.5 sum_c xsq[c, t*128+p] via 8 one-row matmuls (no DMA).
                if li == 0:
                    sqhTs = sqh0Ts_sb[cl]
                else:
                    sqhTs = smpool.tile([128, 8], dt.float32, tag="sqhTs",
                                        name="sqhTs")
                    sqp = ps_uv.tile([128, 16], dt.float32, tag="uvps", name="sqp")
                    for t in range(8):
                        nc.tensor.matmul(sqp[:, 2 * t:2 * t + 2],
                                         xsqs[0][:, t * 128:(t + 1) * 128],
                                         nh2[0:cbs[0], :], start=True, stop=True)
                    nc.scalar.activation(
                        sqhTs[:],
                        sqp[:].rearrange("f (t two) -> f t two", two=2)[:, :, 0],
                        AF.Copy, scale=SCALE)
                yield

                # ---- U/V tables ----
                if packed:
                    vt2 = uvpool.tile([128, N], dt.float32, tag="vt0", name=f"vt2_{cl}")
                    ut2 = uvpool.tile([128, N], dt.float32, tag="ut0", name=f"ut2_{cl}")
                    for j in range(2):
                        sl = slice(j * 512, (j + 1) * 512)
                        vp = ps_uv.tile([64, 512], dt.float32, tag="uvps")
                        for kb, xb in enumerate(xt_blocks):
                            nc.tensor.matmul(
                                vp[:], wav_sb[li][kb * 128: kb * 128 + cbs[kb], :],
                                xb[:, sl], start=(kb == 0), stop=(kb == nkb_in - 1))
                        nc.scalar.activation(vt2[0:64, sl], vp[:], AF.Identity)
                        up = ps_uv.tile([64, 512], dt.float32, tag="uvps")
                        for kb, xb in enumerate(xt_blocks):
                            nc.tensor.matmul(
                                up[:], wau_sb[li][kb * 128: kb * 128 + cbs[kb], :],
                                xb[:, sl], start=(kb == 0), stop=(kb == nkb_in - 1))
                        nc.scalar.activation(ut2[0:64, sl], up[:], AF.Identity,
                                             bias=wba_sb[li][0][:])
                        yield
                    # hi halves via SBUF-SBUF DMA (off the saturated Act):
                    # vt2 duplicated, ut2 shifted left by 64 cols
                    nc.sync.dma_start(vt2[64:128, :], vt2[0:64, :])
                    nc.sync.dma_start(ut2[64:128, 0:960], ut2[0:64, 64:1024])
                    vts, uts = [vt2], [ut2]
                elif nob == 2:
                    # li=3: paired fp16 tables [128, N, 2], plane ob holds
                    # features ob*128..ob*128+127
                    vtp = uvpool.tile([128, N, 2], dt.float16, tag="vt0",
                                      name=f"vtp_{cl}")
                    utp = uvpool.tile([128, N, 2], dt.float16, tag="ut0",
                                      name=f"utp_{cl}")
                    for ob in range(nob):
                        for j in range(2):
                            sl = slice(j * 512, (j + 1) * 512)
                            vp = ps_uv.tile([128, 512], dt.float32, tag="uvps")
                            for kb, xb in enumerate(xt_blocks):
                                nc.tensor.matmul(
                                    vp[:], wav_sb[li][kb * 128: kb * 128 + cbs[kb],
                                                      ob * 128: ob * 128 + 128],
                                    xb[:, sl], start=(kb == 0), stop=(kb == nkb_in - 1))
                            nc.scalar.activation(vtp[:, sl, ob], vp[:], AF.Identity)
                            up = ps_uv.tile([128, 512], dt.float32, tag="uvps")
                            for kb, xb in enumerate(xt_blocks):
                                nc.tensor.matmul(
                                    up[:], wau_sb[li][kb * 128: kb * 128 + cbs[kb],
                                                      ob * 128: ob * 128 + 128],
                                    xb[:, sl], start=(kb == 0), stop=(kb == nkb_in - 1))
                            nc.scalar.activation(utp[:, sl, ob], up[:], AF.Identity,
                                                 bias=wba_sb[li][ob][:])
                            yield
                    vts, uts = [vtp], [utp]
                else:
                    # li=2: single fp32 tables [128, N]
                    vts, uts = [], []
                    for ob in range(nob):
                        Cob = min(128, Co - ob * 128)
                        vt = uvpool.tile([Cob, N], dt.float32, tag=f"vt{ob}")
                        ut = uvpool.tile([Cob, N], dt.float32, tag=f"ut{ob}")
                        for j in range(2):
                            sl = slice(j * 512, (j + 1) * 512)
                            vp = ps_uv.tile([Cob, 512], dt.float32, tag="uvps")
                            for kb, xb in enumerate(xt_blocks):
                                nc.tensor.matmul(
                                    vp[:], wav_sb[li][kb * 128: kb * 128 + cbs[kb],
                                                      ob * 128: ob * 128 + Cob],
                                    xb[:, sl], start=(kb == 0), stop=(kb == nkb_in - 1))
                            nc.scalar.activation(vt[:, sl], vp[:], AF.Identity)
                            up = ps_uv.tile([Cob, 512], dt.float32, tag="uvps")
                            for kb, xb in enumerate(xt_blocks):
                                nc.tensor.matmul(
                                    up[:], wau_sb[li][kb * 128: kb * 128 + cbs[kb],
                                                      ob * 128: ob * 128 + Cob],
                                    xb[:, sl], start=(kb == 0), stop=(kb == nkb_in - 1))
                            nc.scalar.activation(ut[:, sl], up[:], AF.Identity,
                                                 bias=wba_sb[li][ob][0:Cob, :])
                            yield
                        vts.append(vt)
                        uts.append(ut)

                # ---- gram + chunked topk per 128-row block ----
                idx16 = idxpool.tile([128, 8, 20], dt.int16, tag=f"idx16_c{cl}",
                                     name=f"idx16_c{cl}")
                for t in range(8):
                    pk = pks[tctr[0] % 2]
                    tctr[0] += 1
                    pk16 = pk[:].bitcast(dt.int16).rearrange(
                        "f (e two) -> f e two", two=2)
                    for j in range(2):
                        sl = slice(j * 512, (j + 1) * 512)
                        sp = ps_g.tile([128, 512], dt.float32, tag="sps")
                        for kb, xb in enumerate(xt_blocks):
                            nc.tensor.matmul(sp[:], xb[:, t * 128:(t + 1) * 128],
                                             xb[:, sl], start=(kb == 0), stop=False)
                        nc.tensor.matmul(sp[:], ones1[:], sqh[:, sl],
                                         start=False, stop=True)
                        # fp16 score into the high int16 lane of pk
                        nc.scalar.activation(pk16[:, sl, 1].bitcast(dt.float16),
                                             sp[:], AF.Identity, scale=SCALE,
                                             bias=sqhTs[:, t:t + 1])
                        if j == 0:
                            yield
                    cand = smpool.tile([128, 64], dt.float32, tag="cand",
                                       name="cand")
                    pka = pk[:]
                    for c in range(8):
                        nc.vector.max(cand[:, c * 8:(c + 1) * 8],
                                      pka[:, c * 128:(c + 1) * 128])
                    vals = smpool.tile([128, 24], dt.float32, tag="vals")
                    cb_ = smpool.tile([128, 64], dt.float32, tag="cb", name="cb_")
                    cc_ = smpool.tile([128, 64], dt.float32, tag="cc", name="cc_")
                    nc.vector.max(vals[:, 0:8], cand[:])
                    nc.vector.match_replace(cb_[:], vals[:, 0:8], cand[:], -3.0e38)
                    nc.vector.max(vals[:, 8:16], cb_[:])
                    nc.vector.match_replace(cc_[:], vals[:, 8:16], cb_[:], -3.0e38)
                    nc.vector.max(vals[:, 16:24], cc_[:])
                    # low 16 bits of the packed winners = global column index
                    v16 = vals[:, 0:20].bitcast(dt.int16).rearrange(
                        "f (e two) -> f e two", two=2)[:, :, 0]
                    nc.vector.tensor_scalar(idx16[:, t, :], v16, masklo16[:], None,
                                            op0=OP.bitwise_and)
                    yield

                # ---- idxw ----
                load_weights(li + 1)
                if packed:
                    # groups 0-3: idxs of points 0:64 per t-block; 4-7: 64:128
                    idxw = idxpool.tile([128, 8 * 80], dt.int16, tag="idxwp", bufs=3,
                                        name=f"idxwp_c{cl}")
                    vA = idxw[0:16, :].rearrange("p (t c k) -> p t c k", t=8, c=4, k=20)
                    vB = idxw[64:80, :].rearrange("p (t c k) -> p t c k", t=8, c=4, k=20)
                    for th in range(2):
                        ts_ = slice(th * 4, th * 4 + 4)
                        cs = slice(th * 320, th * 320 + 320)
                        for c in range(4):
                            nc.sync.dma_start(vA[:, ts_, c, :],
                                              idx16[16 * c:16 * (c + 1), ts_, :])
                            nc.sync.dma_start(vB[:, ts_, c, :],
                                              idx16[64 + 16 * c:64 + 16 * (c + 1), ts_, :])
                        # doubling tree replication: 2 DMAs per half vs 3
                        nc.sync.dma_start(idxw[16:32, cs], idxw[0:16, cs])
                        nc.sync.dma_start(idxw[32:64, cs], idxw[0:32, cs])
                        nc.sync.dma_start(idxw[80:96, cs], idxw[64:80, cs])
                        nc.sync.dma_start(idxw[96:128, cs], idxw[64:96, cs])
                        yield
                else:
                    idxw = idxpool.tile([128, 8 * 160], dt.int16, tag="idxw", bufs=3,
                                        name=f"idxw_c{cl}")
                    idxw_v = idxw[0:16, :].rearrange("p (t c k) -> p t c k", t=8, c=8, k=20)
                    for th in range(2):
                        ts_ = slice(th * 4, th * 4 + 4)
                        cs = slice(th * 640, th * 640 + 640)
                        for c in range(8):
                            nc.sync.dma_start(idxw_v[:, ts_, c, :],
                                              idx16[16 * c:16 * (c + 1), ts_, :])
                        # doubling tree replication: 3 DMAs vs 7
                        nc.sync.dma_start(idxw[16:32, cs], idxw[0:16, cs])
                        nc.sync.dma_start(idxw[32:64, cs], idxw[0:32, cs])
                        nc.sync.dma_start(idxw[64:128, cs], idxw[0:64, cs])
                        yield
                stateA[(li, cl)] = (vts, uts, idxw)

            poolq = {}

            def phaseB_units(li, cl):
                C, Co = LAYERS[li]
                packed = (Co == 64)
                nob = (Co + 127) // 128
                # previous layer's global-max-pool reduces: ready now, and they
                # fill the DVE gap while this phase's first h2 chain fills
                ready = [k for k in list(poolq)
                         if k[0] * CPC + k[1] < li * CPC + cl]
                for k in ready:
                    for dst, srcap in poolq.pop(k):
                        nc.vector.tensor_reduce(dst, srcap, axis=AX.X, op=OP.max)
                yield
                vts, uts, idxw = stateA.pop((li, cl))
                if packed:
                    vt2, ut2 = vts[0], uts[0]
                    raw = xpool.tile([128, 512], dt.float32, tag="rawp", bufs=2,
                                     name=f"raw{li}_{cl}")
                    for ch in range(8):
                        vg = vgpool.tile([128, ECHUNK], dt.float32, tag="vg0",
                                         name="vg0")
                        nc.gpsimd.ap_gather(vg[:], vt2[:], idxw[:, ch * 80:(ch + 1) * 80],
                                            channels=128, num_elems=N, d=1,
                                            num_idxs=ECHUNK)
                        vgv = vg[:].rearrange("f (c k p) -> f c k p", c=4, k=20, p=16)
                        ub = (ut2[:, ch * 128: ch * 128 + 64]
                              .rearrange("f (c p) -> f c p", c=4)
                              .unsqueeze(2).to_broadcast([128, 4, 20, 16]))
                        # u+v add: mostly Pool, every 4th on DVE (local rate
                        # balance within the B chain)
                        aeng = nc.vector if ch % 2 == 1 else nc.gpsimd
                        aeng.tensor_tensor(vgv, vgv, ub, op=OP.add)
                        h1r = vgpool.tile([128, ECHUNK], dt.float16, tag="h1r0",
                                          name="h1r0")
                        nc.scalar.activation(h1r[:], vg[:], AF.Relu)
                        for sub in range(2):
                            hp = ps_h2.tile([128, 1024], dt.float32, tag="h2ps")
                            for bi in range(2):
                                cols = slice(sub * 640 + bi * 320,
                                             sub * 640 + bi * 320 + 320)
                                nc.tensor.matmul(hp[:, bi * 512: bi * 512 + 320],
                                                 wb_sb[li][0][:],
                                                 h1r[:, cols],
                                                 start=True, stop=True)
                            hv = (hp[:].rearrange("f (b q) -> f b q", b=2)[:, :, 0:320]
                                  .rearrange("f b (k p) -> f b p k", k=20, p=16))
                            pt0 = ch * 64 + sub * 32
                            nc.vector.tensor_reduce(
                                raw[:, pt0:pt0 + 32].rearrange("f (c p) -> f c p", c=2),
                                hv, axis=AX.X, op=OP.max)
                            yield
                    # bias + f32r round, then partition-unshift A/B via DMA
                    xnr = xpool.tile([128, 512], dt.float32r, tag="xnr", bufs=2,
                                     name=f"xnr{li}_{cl}")
                    xnext = xpool.tile([64, N], dt.float32r, tag=f"x{li + 1}_c{cl}",
                                       name=f"x{li + 1}_{cl}")
                    xv = xnext[:].rearrange("f (ch q) -> f ch q", ch=8, q=128)
                    for hh in range(2):
                        csl = slice(hh * 256, hh * 256 + 256)
                        chs = slice(hh * 4, hh * 4 + 4)
                        nc.scalar.activation(xnr[:, csl], raw[:, csl], AF.Identity,
                                             bias=wbb_sb[li][0][:])
                        nc.sync.dma_start(
                            xv[:, chs, 0:64],
                            xnr[0:64, csl].rearrange("f (ch q) -> f ch q", ch=4, q=64))
                        nc.sync.dma_start(
                            xv[:, chs, 64:128],
                            xnr[64:128, csl].rearrange("f (ch q) -> f ch q", ch=4, q=64))
                    xnext_blocks = [xnext]
                    dst = (pooled[0:64, 4 * cl: 4 * cl + 1] if li == 0
                           else pooled[64:128, 4 * cl: 4 * cl + 1])
                    poolq.setdefault((li, cl), []).append(
                        (dst, xnext[:].bitcast(dt.float32)))
                else:
                    xnext_blocks = []
                    raws = []
                    for ob in range(nob):
                        raws.append(xpool.tile([128, N], dt.float32,
                                               tag=f"rawob{ob}", bufs=2,
                                               name=f"raw{li}_{ob}_{cl}"))
                    for ch in range(16):
                        h1s = []
                        if nob == 2:
                            # paired fp16 gather via the fp32-bitcast table:
                            # d=1 fp32 moves the same bytes at half the cost
                            vgp = vgpool.tile([128, ECHUNK, 2], dt.float16,
                                              tag="vg0", name="vgp")
                            nc.gpsimd.ap_gather(vgp[:].bitcast(dt.float32),
                                                vts[0][:].bitcast(dt.float32),
                                                idxw[:, ch * 80:(ch + 1) * 80],
                                                channels=128, num_elems=N, d=1,
                                                num_idxs=ECHUNK)
                            vgv = vgp[:].rearrange("f (c k p) d -> f c k (p d)",
                                                   c=4, k=20, p=16)
                            ub = (uts[0][:, ch * 64: ch * 64 + 64, :]
                                  .rearrange("f (c p) d -> f c (p d)", c=4)
                                  .unsqueeze(2).to_broadcast([128, 4, 20, 32]))
                            aeng = nc.gpsimd if ch % 3 == 2 else nc.vector
                            aeng.tensor_tensor(vgv, vgv, ub, op=OP.add)
                            for ob in range(nob):
                                h1r = vgpool.tile([128, ECHUNK], dt.float16,
                                                  tag=f"h1r{ob}", name=f"h1r{ob}")
                                nc.scalar.activation(h1r[:], vgp[:, :, ob], AF.Relu)
                                h1s.append(h1r)
                        else:
                          for ob in range(nob):
                            vg = vgpool.tile([128, ECHUNK], dt.float32, tag="vg0",
                                             name=f"vg{ob}")
                            nc.gpsimd.ap_gather(vg[:], vts[ob][:],
                                                idxw[:, ch * 80:(ch + 1) * 80],
                                                channels=128, num_elems=N, d=1,
                                                num_idxs=ECHUNK)
                            vgv = vg[:].rearrange("f (c k p) -> f c k p", c=4, k=20, p=16)
                            ub = (uts[ob][:, ch * 64: ch * 64 + 64]
                                  .rearrange("f (c p) -> f c p", c=4)
                                  .unsqueeze(2).to_broadcast([128, 4, 20, 16]))
                            aeng = nc.vector if ch % 2 == 1 else nc.gpsimd
                            aeng.tensor_tensor(vgv, vgv, ub, op=OP.add)
                            h1r = vgpool.tile([128, ECHUNK], dt.float16,
                                              tag=f"h1r{ob}", name=f"h1r{ob}")
                            nc.scalar.activation(h1r[:], vg[:], AF.Relu)
                            h1s.append(h1r)
                        for ob2 in range(nob):
                            for sub in range(2):
                                hp = ps_h2.tile([128, 1024], dt.float32, tag="h2ps")
                                for bi in range(2):
                                    cols = slice(sub * 640 + bi * 320,
                                                 sub * 640 + bi * 320 + 320)
                                    for kb in range(nob):
                                        nc.tensor.matmul(
                                            hp[:, bi * 512: bi * 512 + 320],
                                            wb_sb[li][kb][:, ob2 * 128: ob2 * 128 + 128],
                                            h1s[kb][:, cols],
                                            start=(kb == 0), stop=(kb == nob - 1))
                                hv = (hp[:].rearrange("f (b q) -> f b q", b=2)[:, :, 0:320]
                                      .rearrange("f b (k p) -> f b p k", k=20, p=16))
                                pt0 = ch * 64 + sub * 32
                                nc.vector.tensor_reduce(
                                    raws[ob2][:, pt0:pt0 + 32]
                                    .rearrange("f (c p) -> f c p", c=2),
                                    hv, axis=AX.X, op=OP.max)
                                yield
                    for ob in range(nob):
                        xn = xpool.tile([128, N], dt.float32r,
                                        tag=f"x{li + 1}_{ob}_c{cl}",
                                        name=f"x{li + 1}_{ob}_{cl}")
                        for hh in range(2):
                            csl = slice(hh * 512, hh * 512 + 512)
                            nc.scalar.activation(xn[:, csl], raws[ob][:, csl],
                                                 AF.Identity, bias=wbb_sb[li][ob][:])
                        xnext_blocks.append(xn)
                        if li == 2:
                            dst = pooled[0:128, 4 * cl + 1: 4 * cl + 2]
                        else:
                            dst = pooled[0:128, 4 * cl + 2 + ob: 4 * cl + 3 + ob]
                        poolq.setdefault((li, cl), []).append(
                            (dst, xn[:].bitcast(dt.float32)))
                xts_c[cl].append(xnext_blocks)

            # fine-grained software pipeline: emit units of B(li, cl) and the
            # NEXT phase-slot's A interleaved, so Pool's gather+add chain (B)
            # overlaps DVE's topk chain (A) without head-of-line blocking
            def drain(gen):
                if gen is not None:
                    for _ in gen:
                        pass

            # unit counts per phase (must track the yield structure above)
            def a_count(li):
                return 1 + (4 if li == 3 else 2) + 16 + 2

            def b_count(li):
                return 1 + (16 if LAYERS[li][1] == 64 else
                            16 * 2 * ((LAYERS[li][1] + 127) // 128))

            def zip_emit(bgen, nb_tot, agen, na_tot):
                # fraction-paced interleave: spread A units across B's span
                # so every engine's queue sees independent A work between
                # runs of dependency-blocked B instructions
                if agen is None:
                    drain(bgen)
                    return
                done_b = done_a = False
                nb = na = 0
                while not (done_b and done_a):
                    if not done_b and (done_a or nb * na_tot * 3 <= na * nb_tot * 4):
                        done_b = next(bgen, StopIteration) is StopIteration
                        nb += 1
                    else:
                        done_a = next(agen, StopIteration) is StopIteration
                        na += 1

            drain(phaseA_units(0, 0))
            slots = []
            for li in range(4):
                for cl in range(CPC):
                    slots.append((li, cl))
            for i, (li, cl) in enumerate(slots):
                # A-partner: the slot one step ahead in the pipeline
                nli, ncl = (slots[i + 1] if i + 1 < len(slots) else (None, None))
                agen = phaseA_units(nli, ncl) if nli is not None else None
                zip_emit(phaseB_units(li, cl), b_count(li), agen,
                         a_count(nli) if nli is not None else 0)
            for key in sorted(poolq.keys()):
                for dst, srcap in poolq[key]:
                    nc.vector.tensor_reduce(dst, srcap, axis=AX.X, op=OP.max)
            poolq.clear()
            lin1_sb = []
            for kb in range(4):
                t = cpool.tile([128, 1024], dt.float32, tag=f"lin1_{kb}")
                nc.sync.dma_start(t[:], lin1[kb * 128:(kb + 1) * 128, :])
                lin1_sb.append(t)
            lin1b_sb = cpool.tile([128, 8], dt.float32, tag="lin1b")
            nc.sync.dma_start(lin1b_sb[:], lin1b[:])

            # ---------------- head: h = pooled @ lin1 + lin1_b ----------------
            h_sb = cpool.tile([128, 8, CPC], dt.float32, tag="h_sb")
            for pb_ in range(8):
                hp = ps_g.tile([128, CPC], dt.float32, tag="sps")
                for kb in range(4):
                    rhs = pooled[:, :].rearrange("f (c k) -> f k c", c=CPC)[:, kb, :]
                    nc.tensor.matmul(hp[:], lin1_sb[kb][:, pb_ * 128:(pb_ + 1) * 128],
                                     rhs, start=(kb == 0), stop=(kb == 3))
                nc.scalar.activation(h_sb[:, pb_, :], hp[:], AF.Identity,
                                     bias=lin1b_sb[:, pb_:pb_ + 1])
            nc.sync.dma_start(h_out[:, :], h_sb[:])

    nc.compile()
    _NC_CACHE["nc"] = nc
    return nc


def kernel(**inputs):
    pos = np.asarray(inputs["pos"], np.float32)
    posT = np.ascontiguousarray(pos.reshape(B, N, 3).transpose(0, 2, 1), dtype=np.float32)

    common = {}
    for li in range(4):
        C, Co = LAYERS[li]
        Wa = np.asarray(inputs[f"W{li + 1}a"], np.float32)
        ba = np.asarray(inputs[f"b{li + 1}a"], np.float32)
        Wb_ = np.asarray(inputs[f"W{li + 1}b"], np.float32)
        bb_ = np.asarray(inputs[f"b{li + 1}b"], np.float32)
        common[f"wau{li}"] = np.ascontiguousarray(Wa[:C] - Wa[C:])
        common[f"wav{li}"] = np.ascontiguousarray(Wa[C:])
        common[f"wba{li}"] = np.ascontiguousarray(ba[:, None])
        if Co == 64:
            bd = np.zeros((128, 128), np.float16)
            bd[:64, :64] = Wb_.astype(np.float16)
            bd[64:, 64:] = Wb_.astype(np.float16)
            common[f"wb{li}"] = bd
        else:
            common[f"wb{li}"] = np.ascontiguousarray(Wb_.astype(np.float16))
        common[f"wbb{li}"] = np.ascontiguousarray(bb_[:, None])
    common["lin1"] = np.asarray(inputs["lin1_w"], np.float32)
    common["lin1b"] = np.ascontiguousarray(
        np.asarray(inputs["lin1_b"], np.float32).reshape(8, 128).T)

    nc = _build_nc()
    in_maps = []
    for c in range(NCORES):
        m = dict(common)
        m["posT"] = np.ascontiguousarray(posT[c * CPC:(c + 1) * CPC])
        m["sqh0"] = np.ascontiguousarray(
            -0.5 * (m["posT"] ** 2).sum(1, keepdims=True))
        m["sqh0T"] = np.ascontiguousarray(
            m["sqh0"].reshape(CPC, 8, 128).transpose(0, 2, 1) * SCALE)
        in_maps.append(m)
    res = run_bass_kernel_spmd(nc, in_maps, core_ids=list(range(NCORES)))
    global _LAST_RES
    _LAST_RES = res
    h = np.concatenate(
        [r["h_out"].reshape(128, 8, CPC).transpose(2, 1, 0).reshape(CPC, 1024)
         for r in res.results], 0)   # [16, 1024]

    # host head: BN (cross-batch) + relu + lin2 + log_softmax (fp32)
    gamma = np.asarray(inputs["gamma"], np.float32)
    beta = np.asarray(inputs["beta"], np.float32)
    lin2_w = np.asarray(inputs["lin2_w"], np.float32)
    lin2_b = np.asarray(inputs["lin2_b"], np.float32)
    mu = h.mean(0)
    var = ((h - mu) ** 2).mean(0)
    hn = (h - mu) / np.sqrt(var + 1e-5) * gamma + beta
    hn = np.maximum(hn, 0)
    logits = hn @ lin2_w + lin2_b
    m = logits.max(1, keepdims=True)
    lse = np.log(np.exp(logits - m).sum(1, keepdims=True)) + m
    return (logits - lse).astype(np.float32)


# revision 90
# speedup vs baseline: 1.0182x; 1.0182x over previous
"""DGCNN (4x DynamicEdgeConv + global head) Trainium2 Bass kernel, v3.

Sharding: data-parallel over the 16 clouds -> 8 cores x 2 clouds; tiny BN
head on host (cross-core batch stats).

v3 redesign (vs the 747us v2): the v2 kernel was DVE-bound (95% busy:
top-k Max/MatchReplace/pack 447us + K-max TensorReduce 214us).
  - pack-free top-k: Act evicts gram PSUM scores as fp16 into the HIGH
    int16 lane of a [128, N] fp32 tile whose LOW lanes hold a prewritten
    int16 iota.  The fp32 bit pattern [fp16(score)][iota] is monotone in
    score (float compare; NaN patterns unreachable), so Max/MatchReplace
    run directly on it and the low lane of a winner IS its column index.
    This deletes the DVE pack pass and reuses the evict Act was doing
    anyway.
  - chunked top-k: top-8 per 128-col chunk (8x Max) + top-20-of-64
    second level (Max/MR/Max/MR/Max over 64) instead of 3x Max + 2x
    MatchReplace over the full 1024 cols: 2.1us -> 0.9us per t-block on
    DVE.  (P(>8 of the 20 NN in one 128-chunk) ~ 3e-4 per point; a miss
    swaps a ~20th neighbor for a ~21st.)
  - fp16 U/V tables everywhere; u+v add as fp16 TensorTensor (2x DVE
    rate, or Pool for load balance); relu on Act; L3 gather runs d=1 on
    the fp32-bitcast pair table (half the modeled gather cost).
  - L0/L1 h2 matmuls use a host-built block-diagonal [128,128] Wb so the
    two 64-point groups multiply in one matmul.
"""
import numpy as np

import concourse.bacc as bacc
import concourse.mybir as mybir
import concourse.tile as tile
from concourse.bass_utils import run_bass_kernel_spmd

dt = mybir.dt
AF = mybir.ActivationFunctionType
OP = mybir.AluOpType
AX = mybir.AxisListType

B, N, K = 16, 1024, 20
NCORES = 8
CPC = B // NCORES                    # clouds per core
LAYERS = [(3, 64), (64, 64), (64, 128), (128, 256)]
ECHUNK = 1280                        # idxs per ap_gather
SCALE = 0.0625                       # fp16 score scale (overflow guard)

_NC_CACHE = {}


def _build_nc():
    if "nc" in _NC_CACHE:
        return _NC_CACHE["nc"]
    nc = bacc.Bacc("TRN2", target_bir_lowering=False, debug=False,
                   num_devices=NCORES)

    # ---------------- DRAM I/O ----------------
    posT = nc.dram_tensor("posT", [CPC, 3, N], dt.float32,
                          kind="ExternalInput").ap()
    sqh0_in = nc.dram_tensor("sqh0", [CPC, 1, N], dt.float32,
                             kind="ExternalInput").ap()
    sqh0T_in = nc.dram_tensor("sqh0T", [CPC, 128, 8], dt.float32,
                              kind="ExternalInput").ap()
    wau, wav, wba, wb, wbb = [], [], [], [], []
    for li, (C, Co) in enumerate(LAYERS):
        packed = (Co == 64)
        wau.append(nc.dram_tensor(f"wau{li}", [C, Co], dt.float32, kind="ExternalInput").ap())
        wav.append(nc.dram_tensor(f"wav{li}", [C, Co], dt.float32, kind="ExternalInput").ap())
        wba.append(nc.dram_tensor(f"wba{li}", [Co, 1], dt.float32, kind="ExternalInput").ap())
        wbshape = [128, 128] if packed else [Co, Co]
        wb.append(nc.dram_tensor(f"wb{li}", wbshape, dt.float16, kind="ExternalInput").ap())
        wbb.append(nc.dram_tensor(f"wbb{li}", [Co, 1], dt.float32, kind="ExternalInput").ap())
    lin1 = nc.dram_tensor("lin1", [512, 1024], dt.float32, kind="ExternalInput").ap()
    lin1b = nc.dram_tensor("lin1b", [128, 8], dt.float32, kind="ExternalInput").ap()
    h_out = nc.dram_tensor("h_out", [128, 8 * CPC], dt.float32,
                           kind="ExternalOutput").ap()

    with tile.TileContext(nc) as tc:
        with (
            tc.tile_pool(name="const", bufs=1) as cpool,
            tc.tile_pool(name="xf", bufs=1) as xpool,
            tc.tile_pool(name="uv", bufs=2) as uvpool,
            tc.tile_pool(name="small", bufs=2) as smpool,
            tc.tile_pool(name="idx", bufs=2) as idxpool,
            tc.tile_pool(name="vg", bufs=4) as vgpool,
            tc.tile_pool(name="ps_g", bufs=2, space="PSUM") as ps_g,
            tc.tile_pool(name="ps_uv", bufs=2, space="PSUM") as ps_uv,
            tc.tile_pool(name="ps_h2", bufs=2, space="PSUM") as ps_h2,
        ):
            # ------------- constants -------------
            masklo16 = cpool.tile([128, 1], dt.int16, tag="masklo16")
            nc.vector.memset(masklo16[:], 1023)
            nh0 = cpool.tile([128, 1], dt.float32, tag="nh0")
            nc.vector.memset(nh0[:], -0.5)
            nh = cpool.tile([128, 1], dt.float32r, tag="nh")
            nc.scalar.activation(nh[:], nh0[:], AF.Copy)
            nh2 = cpool.tile([128, 2], dt.float32r, tag="nh2")
            nc.scalar.activation(nh2[:], nh0[:].to_broadcast([128, 2]), AF.Copy)
            ones0 = cpool.tile([1, 128], dt.float32, tag="ones0")
            nc.vector.memset(ones0[:], 1.0)
            ones1 = cpool.tile([1, 128], dt.float32r, tag="ones1")
            nc.scalar.activation(ones1[:], ones0[:], AF.Copy)

            # per-cloud feature tiles (f32r) - load first so layer 0 can start
            xts_c = {}
            sqh_sb = {}
            sqh0Ts_sb = {}
            for cl in range(CPC):
                x0 = xpool.tile([3, N], dt.float32r, tag=f"x0_c{cl}", name=f"x0c{cl}")
                nc.sync.dma_start(x0.copy()[:], posT[cl].bitcast(dt.float32r))
                xts_c[cl] = [[x0]]
                s0 = smpool.tile([1, N], dt.float32r, tag=f"sqh_c{cl}", bufs=1,
                                 name=f"sqh0c{cl}")
                nc.sync.dma_start(s0.copy()[:], sqh0_in[cl].bitcast(dt.float32r))
                sqh_sb[cl] = s0
                s0T = smpool.tile([128, 8], dt.float32, tag=f"sqh0Ts_c{cl}",
                                  bufs=1, name=f"sqh0Tc{cl}")
                nc.sync.dma_start(s0T.copy()[:], sqh0T_in[cl])
                sqh0Ts_sb[cl] = s0T

            # packed-score tiles: low int16 lanes = iota (written on-chip by
            # the otherwise-idle Pool engine), high lanes get fp16 scores
            # from Act.  Two buffers, rotated per t.
            pks = []
            for pb in range(2):
                pk = cpool.tile([128, N], dt.float32, tag=f"pk{pb}",
                                name=f"pk{pb}")
                pk16i = pk[:].bitcast(dt.int16).rearrange(
                    "f (e two) -> f e two", two=2)
                nc.gpsimd.iota(pk16i[:, :, 0], pattern=[[1, N]], base=0,
                               channel_multiplier=0)
                pks.append(pk)

            # weight tiles; DMAs are deferred to one layer ahead of use so
            # the startup HWDGE queue doesn't gate the first topk
            wau_sb, wav_sb, wba_sb, wb_sb, wbb_sb = [], [], [], [], []
            wload = []
            for li, (C, Co) in enumerate(LAYERS):
                packed = (Co == 64)
                dmas = []
                t = cpool.tile([C, Co], dt.float32r, tag=f"wau{li}")
                dmas.append((t[:], wau[li][:].bitcast(dt.float32r))); wau_sb.append(t)
                t = cpool.tile([C, Co], dt.float32r, tag=f"wav{li}")
                dmas.append((t[:], wav[li][:].bitcast(dt.float32r))); wav_sb.append(t)
                nob = (Co + 127) // 128
                bblks = []
                for ob in range(nob):
                    kk = min(128, Co - ob * 128)
                    t = cpool.tile([kk, 1], dt.float32, tag=f"wba{li}_{ob}")
                    dmas.append((t[:], wba[li][ob * 128: ob * 128 + kk, :]))
                    bblks.append(t)
                wba_sb.append(bblks)
                if li == 0:
                    # layer-0 UV weights must load upfront; its B weights can
                    # wait until the A(0,0) idxw unit with the layer-1 batch
                    wload.append(list(dmas))
                    dmas.clear()
                if packed:
                    # host-built block-diagonal [128, 128]
                    t = cpool.tile([128, 128], dt.float16, tag=f"wb{li}")
                    dmas.append((t[:], wb[li][:]))
                    wb_sb.append([t])
                    t = cpool.tile([128, 1], dt.float32, tag=f"wbb{li}")
                    dmas.append((t[0:64, :], wbb[li][:]))
                    dmas.append((t[64:128, :], wbb[li][:]))
                    wbb_sb.append([t])
                else:
                    nkb = (Co + 127) // 128
                    blks = []
                    for kb in range(nkb):
                        kk = min(128, Co - kb * 128)
                        t = cpool.tile([kk, Co], dt.float16, tag=f"wb{li}_{kb}")
                        dmas.append((t[:], wb[li][kb * 128: kb * 128 + kk, :]))
                        blks.append(t)
                    wb_sb.append(blks)
                    bblks = []
                    for ob in range(nob):
                        kk = min(128, Co - ob * 128)
                        t = cpool.tile([kk, 1], dt.float32, tag=f"wbb{li}_{ob}")
                        dmas.append((t[:], wbb[li][ob * 128: ob * 128 + kk, :]))
                        bblks.append(t)
                    wbb_sb.append(bblks)
                wload.append(dmas)

            # wload groups: [UV0], [B0], [L1], [L2], [L3]
            _wsched = {0: [0], 1: [1, 2], 2: [3], 3: [4], 4: []}
            wloaded = set()

            def load_weights(step):
                for g in _wsched.get(step, []):
                    if g in wloaded or g >= len(wload):
                        continue
                    wloaded.add(g)
                    for dst, src in wload[g]:
                        nc.sync.dma_start(dst, src)

            load_weights(0)
            pooled = cpool.tile([128, 4 * CPC], dt.float32, tag="pooled")

            stateA = {}
            tctr = [0]          # global t-block counter for pk rotation

            def phaseA_units(li, cl):
                C, Co = LAYERS[li]
                packed = (Co == 64)
                nob = (Co + 127) // 128
                xt_blocks = xts_c[cl][li]      # input blocks [cb, N] f32r
                nkb_in = len(xt_blocks)
                cbs = [min(128, C - kb * 128) for kb in range(nkb_in)]
                # ---- sq pipeline: sqh = -0.5|x|^2 (f32r) ----
                if li == 0:
                    sqh = sqh_sb[cl]
                else:
                    xsqs = []
                    for kb, xb in enumerate(xt_blocks):
                        xsq = smpool.tile([cbs[kb], N], dt.float32r, tag=f"xsq{kb}",
                                          name=f"xsq{kb}")
                        nc.scalar.activation(xsq[:], xb[:].bitcast(dt.float32), AF.Square)
                        xsqs.append(xsq)
                    sq_ps = []
                    for j in range(2):
                        sl = slice(j * 512, (j + 1) * 512)
                        sp = ps_uv.tile([1, 512], dt.float32, tag="uvps", name=f"sqps{j}")
                        for kb, xsq in enumerate(xsqs):
                            nc.tensor.matmul(sp[:], nh[0:cbs[kb], :], xsq[:, sl],
                                             start=(kb == 0), stop=(kb == nkb_in - 1))
                        sq_ps.append(sp)
                    sqh = smpool.tile([1, N], dt.float32r, tag=f"sqh_c{cl}", bufs=1,
                                      name=f"sqh{li}_{cl}")
                    for j in range(2):
                        sl = slice(j * 512, (j + 1) * 512)
                        nc.scalar.activation(sqh[:, sl], sq_ps[j][:], AF.Identity)
                # per-row bias = sqh_i * SCALE so the evicted fp16 score is
                # ~ -d_ij/2 * SCALE: top scores sit near 0 where fp16
                # resolution is relative to each distance.  sqhT[p, t] =
                # -0.5 sum_c xsq[c, t*128+p] via 8 one-row matmuls (no DMA).
                if li == 0:
                    sqhTs = sqh0Ts_sb[cl]
                else:
                    sqhTs = smpool.tile([128, 8], dt.float32, tag="sqhTs",
                                        name="sqhTs")
                    sqp = ps_uv.tile([128, 16], dt.float32, tag="uvps", name="sqp")
                    for t in range(8):
                        nc.tensor.matmul(sqp[:, 2 * t:2 * t + 2],
                                         xsqs[0][:, t * 128:(t + 1) * 128],
                                         nh2[0:cbs[0], :], start=True, stop=True)
                    nc.scalar.activation(
                        sqhTs[:],
                        sqp[:].rearrange("f (t two) -> f t two", two=2)[:, :, 0],
                        AF.Copy, scale=SCALE)
                yield

                # ---- U/V tables ----
                if packed:
                    vt2 = uvpool.tile([128, N], dt.float32, tag="vt0", name=f"vt2_{cl}")
                    ut2 = uvpool.tile([128, N], dt.float32, tag="ut0", name=f"ut2_{cl}")
                    for j in range(2):
                        sl = slice(j * 512, (j + 1) * 512)
                        vp = ps_uv.tile([64, 512], dt.float32, tag="uvps")
                        for kb, xb in enumerate(xt_blocks):
                            nc.tensor.matmul(
                                vp[:], wav_sb[li][kb * 128: kb * 128 + cbs[kb], :],
                                xb[:, sl], start=(kb == 0), stop=(kb == nkb_in - 1))
                        nc.scalar.activation(vt2[0:64, sl], vp[:], AF.Identity)
                        up = ps_uv.tile([64, 512], dt.float32, tag="uvps")
                        for kb, xb in enumerate(xt_blocks):
                            nc.tensor.matmul(
                                up[:], wau_sb[li][kb * 128: kb * 128 + cbs[kb], :],
                                xb[:, sl], start=(kb == 0), stop=(kb == nkb_in - 1))
                        nc.scalar.activation(ut2[0:64, sl], up[:], AF.Identity,
                                             bias=wba_sb[li][0][:])
                        yield
                    # hi halves via SBUF-SBUF DMA (off the saturated Act):
                    # vt2 duplicated, ut2 shifted left by 64 cols
                    nc.sync.dma_start(vt2[64:128, :], vt2[0:64, :])
                    nc.sync.dma_start(ut2[64:128, 0:960], ut2[0:64, 64:1024])
                    vts, uts = [vt2], [ut2]
                elif nob == 2:
                    # li=3: paired fp16 tables [128, N, 2], plane ob holds
                    # features ob*128..ob*128+127
                    vtp = uvpool.tile([128, N, 2], dt.float16, tag="vt0",
                                      name=f"vtp_{cl}")
                    utp = uvpool.tile([128, N, 2], dt.float16, tag="ut0",
                                      name=f"utp_{cl}")
                    for ob in range(nob):
                        for j in range(2):
                            sl = slice(j * 512, (j + 1) * 512)
                            vp = ps_uv.tile([128, 512], dt.float32, tag="uvps")
                            for kb, xb in enumerate(xt_blocks):
                                nc.tensor.matmul(
                                    vp[:], wav_sb[li][kb * 128: kb * 128 + cbs[kb],
                                                      ob * 128: ob * 128 + 128],
                                    xb[:, sl], start=(kb == 0), stop=(kb == nkb_in - 1))
                            nc.scalar.activation(vtp[:, sl, ob], vp[:], AF.Identity)
                            up = ps_uv.tile([128, 512], dt.float32, tag="uvps")
                            for kb, xb in enumerate(xt_blocks):
                                nc.tensor.matmul(
                                    up[:], wau_sb[li][kb * 128: kb * 128 + cbs[kb],
                                                      ob * 128: ob * 128 + 128],
                                    xb[:, sl], start=(kb == 0), stop=(kb == nkb_in - 1))
                            nc.scalar.activation(utp[:, sl, ob], up[:], AF.Identity,
                                                 bias=wba_sb[li][ob][:])
                            yield
                    vts, uts = [vtp], [utp]
                else:
                    # li=2: single fp32 tables [128, N]
                    vts, uts = [], []
                    for ob in range(nob):
                        Cob = min(128, Co - ob * 128)
                        vt = uvpool.tile([Cob, N], dt.float32, tag=f"vt{ob}")
                        ut = uvpool.tile([Cob, N], dt.float32, tag=f"ut{ob}")
                        for j in range(2):
                            sl = slice(j * 512, (j + 1) * 512)
                            vp = ps_uv.tile([Cob, 512], dt.float32, tag="uvps")
                            for kb, xb in enumerate(xt_blocks):
                                nc.tensor.matmul(
                                    vp[:], wav_sb[li][kb * 128: kb * 128 + cbs[kb],
                                                      ob * 128: ob * 128 + Cob],
                                    xb[:, sl], start=(kb == 0), stop=(kb == nkb_in - 1))
                            nc.scalar.activation(vt[:, sl], vp[:], AF.Identity)
                            up = ps_uv.tile([Cob, 512], dt.float32, tag="uvps")
                            for kb, xb in enumerate(xt_blocks):
                                nc.tensor.matmul(
                                    up[:], wau_sb[li][kb * 128: kb * 128 + cbs[kb],
                                                      ob * 128: ob * 128 + Cob],
                                    xb[:, sl], start=(kb == 0), stop=(kb == nkb_in - 1))
                            nc.scalar.activation(ut[:, sl], up[:], AF.Identity,
                                                 bias=wba_sb[li][ob][0:Cob, :])
                            yield
                        vts.append(vt)
                        uts.append(ut)

                # ---- gram + chunked topk per 128-row block ----
                idx16 = idxpool.tile([128, 8, 20], dt.int16, tag=f"idx16_c{cl}",
                                     name=f"idx16_c{cl}")
                for t in range(8):
                    pk = pks[tctr[0] % 2]
                    tctr[0] += 1
                    pk16 = pk[:].bitcast(dt.int16).rearrange(
                        "f (e two) -> f e two", two=2)
                    for j in range(2):
                        sl = slice(j * 512, (j + 1) * 512)
                        sp = ps_g.tile([128, 512], dt.float32, tag="sps")
                        for kb, xb in enumerate(xt_blocks):
                            nc.tensor.matmul(sp[:], xb[:, t * 128:(t + 1) * 128],
                                             xb[:, sl], start=(kb == 0), stop=False)
                        nc.tensor.matmul(sp[:], ones1[:], sqh[:, sl],
                                         start=False, stop=True)
                        # fp16 score into the high int16 lane of pk
                        nc.scalar.activation(pk16[:, sl, 1].bitcast(dt.float16),
                                             sp[:], AF.Identity, scale=SCALE,
                                             bias=sqhTs[:, t:t + 1])
                        if j == 0:
                            yield
                    cand = smpool.tile([128, 64], dt.float32, tag="cand",
                                       name="cand")
                    pka = pk[:]
                    for c in range(8):
                        nc.vector.max(cand[:, c * 8:(c + 1) * 8],
                                      pka[:, c * 128:(c + 1) * 128])
                    vals = smpool.tile([128, 24], dt.float32, tag="vals")
                    cb_ = smpool.tile([128, 64], dt.float32, tag="cb", name="cb_")
                    cc_ = smpool.tile([128, 64], dt.float32, tag="cc", name="cc_")
                    nc.vector.max(vals[:, 0:8], cand[:])
                    nc.vector.match_replace(cb_[:], vals[:, 0:8], cand[:], -3.0e38)
                    nc.vector.max(vals[:, 8:16], cb_[:])
                    nc.vector.match_replace(cc_[:], vals[:, 8:16], cb_[:], -3.0e38)
                    nc.vector.max(vals[:, 16:24], cc_[:])
                    # low 16 bits of the packed winners = global column index
                    v16 = vals[:, 0:20].bitcast(dt.int16).rearrange(
                        "f (e two) -> f e two", two=2)[:, :, 0]
                    nc.vector.tensor_scalar(idx16[:, t, :], v16, masklo16[:], None,
                                            op0=OP.bitwise_and)
                    yield

                # ---- idxw ----
                load_weights(li + 1)
                if packed:
                    # groups 0-3: idxs of points 0:64 per t-block; 4-7: 64:128
                    idxw = idxpool.tile([128, 8 * 80], dt.int16, tag="idxwp", bufs=3,
                                        name=f"idxwp_c{cl}")
                    vA = idxw[0:16, :].rearrange("p (t c k) -> p t c k", t=8, c=4, k=20)
                    vB = idxw[64:80, :].rearrange("p (t c k) -> p t c k", t=8, c=4, k=20)
                    for th in range(2):
                        ts_ = slice(th * 4, th * 4 + 4)
                        cs = slice(th * 320, th * 320 + 320)
                        for c in range(4):
                            nc.sync.dma_start(vA[:, ts_, c, :],
                                              idx16[16 * c:16 * (c + 1), ts_, :])
                            nc.sync.dma_start(vB[:, ts_, c, :],
                                              idx16[64 + 16 * c:64 + 16 * (c + 1), ts_, :])
                        # doubling tree replication: 2 DMAs per half vs 3
                        nc.sync.dma_start(idxw[16:32, cs], idxw[0:16, cs])
                        nc.sync.dma_start(idxw[32:64, cs], idxw[0:32, cs])
                        nc.sync.dma_start(idxw[80:96, cs], idxw[64:80, cs])
                        nc.sync.dma_start(idxw[96:128, cs], idxw[64:96, cs])
                        yield
                else:
                    idxw = idxpool.tile([128, 8 * 160], dt.int16, tag="idxw", bufs=3,
                                        name=f"idxw_c{cl}")
                    idxw_v = idxw[0:16, :].rearrange("p (t c k) -> p t c k", t=8, c=8, k=20)
                    for th in range(2):
                        ts_ = slice(th * 4, th * 4 + 4)
                        cs = slice(th * 640, th * 640 + 640)
                        for c in range(8):
                            nc.sync.dma_start(idxw_v[:, ts_, c, :],
                                              idx16[16 * c:16 * (c + 1), ts_, :])
                        # doubling tree replication: 3 DMAs vs 7
                        nc.sync.dma_start(idxw[16:32, cs], idxw[0:16, cs])
                        nc.sync.dma_start(idxw[32:64, cs], idxw[0:32, cs])
                        nc.sync.dma_start(idxw[64:128, cs], idxw[0:64, cs])
                        yield
                stateA[(li, cl)] = (vts, uts, idxw)

            poolq = {}

            def phaseB_units(li, cl):
                C, Co = LAYERS[li]
                packed = (Co == 64)
                nob = (Co + 127) // 128
                # previous layer's global-max-pool reduces: ready now, and they
                # fill the DVE gap while this phase's first h2 chain fills
                ready = [k for k in list(poolq)
                         if k[0] * CPC + k[1] < li * CPC + cl]
                for k in ready:
                    for dst, srcap in poolq.pop(k):
                        nc.vector.tensor_reduce(dst, srcap, axis=AX.X, op=OP.max)
                yield
                vts, uts, idxw = stateA.pop((li, cl))
                if packed:
                    vt2, ut2 = vts[0], uts[0]
                    raw = xpool.tile([128, 512], dt.float32, tag="rawp", bufs=2,
                                     name=f"raw{li}_{cl}")
                    for ch in range(8):
                        vg = vgpool.tile([128, ECHUNK], dt.float32, tag="vg0",
                                         name="vg0")
                        nc.gpsimd.ap_gather(vg[:], vt2[:], idxw[:, ch * 80:(ch + 1) * 80],
                                            channels=128, num_elems=N, d=1,
                                            num_idxs=ECHUNK)
                        vgv = vg[:].rearrange("f (c k p) -> f c k p", c=4, k=20, p=16)
                        ub = (ut2[:, ch * 128: ch * 128 + 64]
                              .rearrange("f (c p) -> f c p", c=4)
                              .unsqueeze(2).to_broadcast([128, 4, 20, 16]))
                        # u+v add: mostly Pool, every 4th on DVE (local rate
                        # balance within the B chain)
                        aeng = nc.vector if ch % 3 == 2 else nc.gpsimd
                        aeng.tensor_tensor(vgv, vgv, ub, op=OP.add)
                        h1r = vgpool.tile([128, ECHUNK], dt.float16, tag="h1r0",
                                          name="h1r0")
                        nc.scalar.activation(h1r[:], vg[:], AF.Relu)
                        for sub in range(2):
                            hp = ps_h2.tile([128, 1024], dt.float32, tag="h2ps")
                            for bi in range(2):
                                cols = slice(sub * 640 + bi * 320,
                                             sub * 640 + bi * 320 + 320)
                                nc.tensor.matmul(hp[:, bi * 512: bi * 512 + 320],
                                                 wb_sb[li][0][:],
                                                 h1r[:, cols],
                                                 start=True, stop=True)
                            hv = (hp[:].rearrange("f (b q) -> f b q", b=2)[:, :, 0:320]
                                  .rearrange("f b (k p) -> f b p k", k=20, p=16))
                            pt0 = ch * 64 + sub * 32
                            nc.vector.tensor_reduce(
                                raw[:, pt0:pt0 + 32].rearrange("f (c p) -> f c p", c=2),
                                hv, axis=AX.X, op=OP.max)
                            yield
                    # bias + f32r round, then partition-unshift A/B via DMA
                    xnr = xpool.tile([128, 512], dt.float32r, tag="xnr", bufs=2,
                                     name=f"xnr{li}_{cl}")
                    xnext = xpool.tile([64, N], dt.float32r, tag=f"x{li + 1}_c{cl}",
                                       name=f"x{li + 1}_{cl}")
                    xv = xnext[:].rearrange("f (ch q) -> f ch q", ch=8, q=128)
                    for hh in range(2):
                        csl = slice(hh * 256, hh * 256 + 256)
                        chs = slice(hh * 4, hh * 4 + 4)
                        nc.scalar.activation(xnr[:, csl], raw[:, csl], AF.Identity,
                                             bias=wbb_sb[li][0][:])
                        nc.sync.dma_start(
                            xv[:, chs, 0:64],
                            xnr[0:64, csl].rearrange("f (ch q) -> f ch q", ch=4, q=64))
                        nc.sync.dma_start(
                            xv[:, chs, 64:128],
                            xnr[64:128, csl].rearrange("f (ch q) -> f ch q", ch=4, q=64))
                    xnext_blocks = [xnext]
                    dst = (pooled[0:64, 4 * cl: 4 * cl + 1] if li == 0
                           else pooled[64:128, 4 * cl: 4 * cl + 1])
                    poolq.setdefault((li, cl), []).append(
                        (dst, xnext[:].bitcast(dt.float32)))
                else:
                    xnext_blocks = []
                    raws = []
                    for ob in range(nob):
                        raws.append(xpool.tile([128, N], dt.float32,
                                               tag=f"rawob{ob}", bufs=2,
                                               name=f"raw{li}_{ob}_{cl}"))
                    for ch in range(16):
                        h1s = []
                        if nob == 2:
                            # paired fp16 gather via the fp32-bitcast table:
                            # d=1 fp32 moves the same bytes at half the cost
                            vgp = vgpool.tile([128, ECHUNK, 2], dt.float16,
                                              tag="vg0", name="vgp")
                            nc.gpsimd.ap_gather(vgp[:].bitcast(dt.float32),
                                                vts[0][:].bitcast(dt.float32),
                                                idxw[:, ch * 80:(ch + 1) * 80],
                                                channels=128, num_elems=N, d=1,
                                                num_idxs=ECHUNK)
                            vgv = vgp[:].rearrange("f (c k p) d -> f c k (p d)",
                                                   c=4, k=20, p=16)
                            ub = (uts[0][:, ch * 64: ch * 64 + 64, :]
                                  .rearrange("f (c p) d -> f c (p d)", c=4)
                                  .unsqueeze(2).to_broadcast([128, 4, 20, 32]))
                            aeng = nc.gpsimd if ch % 3 == 2 else nc.vector
                            aeng.tensor_tensor(vgv, vgv, ub, op=OP.add)
                            for ob in range(nob):
                                h1r = vgpool.tile([128, ECHUNK], dt.float16,
                                                  tag=f"h1r{ob}", name=f"h1r{ob}")
                                nc.scalar.activation(h1r[:], vgp[:, :, ob], AF.Relu)
                                h1s.append(h1r)
                        else:
                          for ob in range(nob):
                            vg = vgpool.tile([128, ECHUNK], dt.float32, tag="vg0",
                                             name=f"vg{ob}")
                            nc.gpsimd.ap_gather(vg[:], vts[ob][:],
                                                idxw[:, ch * 80:(ch + 1) * 80],
                                                channels=128, num_elems=N, d=1,
                                                num_idxs=ECHUNK)
                            vgv = vg[:].rearrange("f (c k p) -> f c k p", c=4, k=20, p=16)
                            ub = (uts[ob][:, ch * 64: ch * 64 + 64]
                                  .rearrange("f (c p) -> f c p", c=4)
                                  .unsqueeze(2).to_broadcast([128, 4, 20, 16]))
                            aeng = nc.vector if ch % 2 == 1 else nc.gpsimd
                            aeng.tensor_tensor(vgv, vgv, ub, op=OP.add)
                            h1r = vgpool.tile([128, ECHUNK], dt.float16,
                                              tag=f"h1r{ob}", name=f"h1r{ob}")
                            nc.scalar.activation(h1r[:], vg[:], AF.Relu)
                            h1s.append(h1r)
                        for ob2 in range(nob):
                            for sub in range(2):
                                hp = ps_h2.tile([128, 1024], dt.float32, tag="h2ps")
                                for bi in range(2):
                                    cols = slice(sub * 640 + bi * 320,
                                                 sub * 640 + bi * 320 + 320)
                                    for kb in range(nob):
                                        nc.tensor.matmul(
                                            hp[:, bi * 512: bi * 512 + 320],
                                            wb_sb[li][kb][:, ob2 * 128: ob2 * 128 + 128],
                                            h1s[kb][:, cols],
                                            start=(kb == 0), stop=(kb == nob - 1))
                                hv = (hp[:].rearrange("f (b q) -> f b q", b=2)[:, :, 0:320]
                                      .rearrange("f b (k p) -> f b p k", k=20, p=16))
                                pt0 = ch * 64 + sub * 32
                                nc.vector.tensor_reduce(
                                    raws[ob2][:, pt0:pt0 + 32]
                                    .rearrange("f (c p) -> f c p", c=2),
                                    hv, axis=AX.X, op=OP.max)
                                yield
                    for ob in range(nob):
                        xn = xpool.tile([128, N], dt.float32r,
                                        tag=f"x{li + 1}_{ob}_c{cl}",
                                        name=f"x{li + 1}_{ob}_{cl}")
                        for hh in range(2):
                            csl = slice(hh * 512, hh * 512 + 512)
                            nc.scalar.activation(xn[:, csl], raws[ob][:, csl],
                                                 AF.Identity, bias=wbb_sb[li][ob][:])
                        xnext_blocks.append(xn)
                        if li == 2:
                            dst = pooled[0:128, 4 * cl + 1: 4 * cl + 2]
                        else:
                            dst = pooled[0:128, 4 * cl + 2 + ob: 4 * cl + 3 + ob]
                        poolq.setdefault((li, cl), []).append(
                            (dst, xn[:].bitcast(dt.float32)))
                xts_c[cl].append(xnext_blocks)

            # fine-grained software pipeline: emit units of B(li, cl) and the
            # NEXT phase-slot's A interleaved, so Pool's gather+add chain (B)
            # overlaps DVE's topk chain (A) without head-of-line blocking
            def drain(gen):
                if gen is not None:
                    for _ in gen:
                        pass

            # unit counts per phase (must track the yield structure above)
            def a_count(li):
                return 1 + (4 if li == 3 else 2) + 16 + 2

            def b_count(li):
                return 1 + (16 if LAYERS[li][1] == 64 else
                            16 * 2 * ((LAYERS[li][1] + 127) // 128))

            def zip_emit(bgen, nb_tot, agen, na_tot):
                # fraction-paced interleave: spread A units across B's span
                # so every engine's queue sees independent A work between
                # runs of dependency-blocked B instructions
                if agen is None:
                    drain(bgen)
                    return
                done_b = done_a = False
                nb = na = 0
                while not (done_b and done_a):
                    if not done_b and (done_a or nb * na_tot * 3 <= na * nb_tot * 4):
                        done_b = next(bgen, StopIteration) is StopIteration
                        nb += 1
                    else:
                        done_a = next(agen, StopIteration) is StopIteration
                        na += 1

            drain(phaseA_units(0, 0))
            slots = []
            for li in range(4):
                for cl in range(CPC):
                    slots.append((li, cl))
            for i, (li, cl) in enumerate(slots):
                # A-partner: the slot one step ahead in the pipeline
                nli, ncl = (slots[i + 1] if i + 1 < len(slots) else (None, None))
                agen = phaseA_units(nli, ncl) if nli is not None else None
                zip_emit(phaseB_units(li, cl), b_count(li), agen,
                         a_count(nli) if nli is not None else 0)
            for key in sorted(poolq.keys()):
                for dst, srcap in poolq[key]:
                    nc.vector.tensor_reduce(dst, srcap, axis=AX.X, op=OP.max)
            poolq.clear()
            lin1_sb = []
            for kb in range(4):
                t = cpool.tile([128, 1024], dt.float32, tag=f"lin1_{kb}")
                nc.sync.dma_start(t[:], lin1[kb * 128:(kb + 1) * 128, :])
                lin1_sb.append(t)
            lin1b_sb = cpool.tile([128, 8], dt.float32, tag="lin1b")
            nc.sync.dma_start(lin1b_sb[:], lin1b[:])

            # ---------------- head: h = pooled @ lin1 + lin1_b ----------------
            h_sb = cpool.tile([128, 8, CPC], dt.float32, tag="h_sb")
            for pb_ in range(8):
                hp = ps_g.tile([128, CPC], dt.float32, tag="sps")
                for kb in range(4):
                    rhs = pooled[:, :].rearrange("f (c k) -> f k c", c=CPC)[:, kb, :]
                    nc.tensor.matmul(hp[:], lin1_sb[kb][:, pb_ * 128:(pb_ + 1) * 128],
                                     rhs, start=(kb == 0), stop=(kb == 3))
                nc.scalar.activation(h_sb[:, pb_, :], hp[:], AF.Identity,
                                     bias=lin1b_sb[:, pb_:pb_ + 1])
            nc.sync.dma_start(h_out[:, :], h_sb[:])

    nc.compile()
    _NC_CACHE["nc"] = nc
    return nc


def kernel(**inputs):
    pos = np.asarray(inputs["pos"], np.float32)
    posT = np.ascontiguousarray(pos.reshape(B, N, 3).transpose(0, 2, 1), dtype=np.float32)

    common = {}
    for li in range(4):
        C, Co = LAYERS[li]
        Wa = np.asarray(inputs[f"W{li + 1}a"], np.float32)
        ba = np.asarray(inputs[f"b{li + 1}a"], np.float32)
        Wb_ = np.asarray(inputs[f"W{li + 1}b"], np.float32)
        bb_ = np.asarray(inputs[f"b{li + 1}b"], np.float32)
        common[f"wau{li}"] = np.ascontiguousarray(Wa[:C] - Wa[C:])
        common[f"wav{li}"] = np.ascontiguousarray(Wa[C:])
        common[f"wba{li}"] = np.ascontiguousarray(ba[:, None])
        if Co == 64:
            bd = np.zeros((128, 128), np.float16)
            bd[:64, :64] = Wb_.astype(np.float16)
            bd[64:, 64:] = Wb_.astype(np.float16)
            common[f"wb{li}"] = bd
        else:
            common[f"wb{li}"] = np.ascontiguousarray(Wb_.astype(np.float16))
        common[f"wbb{li}"] = np.ascontiguousarray(bb_[:, None])
    common["lin1"] = np.asarray(inputs["lin1_w"], np.float32)
    common["lin1b"] = np.ascontiguousarray(
        np.asarray(inputs["lin1_b"], np.float32).reshape(8, 128).T)

    nc = _build_nc()
    in_maps = []
    for c in range(NCORES):
        m = dict(common)
        m["posT"] = np.ascontiguousarray(posT[c * CPC:(c + 1) * CPC])
        m["sqh0"] = np.ascontiguousarray(
            -0.5 * (m["posT"] ** 2).sum(1, keepdims=True))
        m["sqh0T"] = np.ascontiguousarray(
            m["sqh0"].reshape(CPC, 8, 128).transpose(0, 2, 1) * SCALE)
        in_maps.append(m)
    res = run_bass_kernel_spmd(nc, in_maps, core_ids=list(range(NCORES)))
    global _LAST_RES
    _LAST_RES = res
    h = np.concatenate(
        [r["h_out"].reshape(128, 8, CPC).transpose(2, 1, 0).reshape(CPC, 1024)
         for r in res.results], 0)   # [16, 1024]

    # host head: BN (cross-batch) + relu + lin2 + log_softmax (fp32)
    gamma = np.asarray(inputs["gamma"], np.float32)
    beta = np.asarray(inputs["beta"], np.float32)
    lin2_w = np.asarray(inputs["lin2_w"], np.float32)
    lin2_b = np.asarray(inputs["lin2_b"], np.float32)
    mu = h.mean(0)
    var = ((h - mu) ** 2).mean(0)
    hn = (h - mu) / np.sqrt(var + 1e-5) * gamma + beta
    hn = np.maximum(hn, 0)
    logits = hn @ lin2_w + lin2_b
    m = logits.max(1, keepdims=True)
    lse = np.log(np.exp(logits - m).sum(1, keepdims=True)) + m
    return (logits - lse).astype(np.float32)
